# revision 35
# baseline (speedup 1.0000x reference)
"""DLRM pairwise-interaction kernel for Trainium2 (Bass/Tile), 8-core data parallel.

Problem: inputs [B=8192, N=64, D=128] fp32 ->
         out [B, 2016] fp32 = strictly-lower-tri (i-major) of per-sample Gram
         Z_b = X_b @ X_b^T.

Sharding: pure data parallel, B split into 8 shards of 1024 samples.

v2 design (bf16), per core (1024 samples = 4 groups of 256 = 128 pairs),
software-pipelined so group g's back half interleaves with group g+1's
front half:
  Front half, per 64-sample block (32 pairs):
  - SWDGE loads the block as [128p=(a,n), 32 pairs x 128 d] casting
    fp32->bf16 (two half-block DMAs to cut the fill latency).
  - PE transposes each pair chunk [128,128] -> PSUM (8 per PSUM tile), one
    [128,1024] copy -> xt SBUF (XT layout [d, pair*(a,n)]).
  - Per pair, TWO 64-col Gram matmuls (lhsT=rhs=that sample's 64 XT cols),
    a=0 -> PSUM partitions 0:64, a=1 -> 64:128: zero garbage columns.
    8 pairs per [128,512] fp32 PSUM tile, one straight copy (cast to bf16)
    -> zbig SBUF [(a,i), q*64+j] (q-major).
  Back half, per group (4 parts interleaved with the next group's blocks):
  - 64 j-transposes: zbig[:, :, j] [128=(a,i), 128 q] -> PSUM [128 q, (a,i)],
    4 per PSUM tile, copies -> zs [q-part, (j, a, i)]: sample-major Z.
  - 63 pack copies (row i: [p, a:2, j:i] strided->contig, bf16->fp32 cast)
    -> zp [q, (a, 2016)]: packed tril rows, emitted as soon as their
    j-range of zs is complete.
  - 2 output DMAs (one per a): [128 rows x 8064B contiguous] -> out.
  DMA instruction count per core: 32 in + 8 out (vs ~520 small strided
  DMAs in v1, which serialized on the ~630ns/instr shared HWDGE block and
  wasted DMA bandwidth on sub-512B bursts).
Copies are spread across DVE/Act/Pool; measured ~161-165us/core on HW
(vs 478us for v1), cost-model floor ~116us (33.5MB fp32 in + 8.25MB out
at 360GB/s).
"""

import numpy as np

import concourse.bass as bass
from concourse import bacc, tile, mybir
from concourse import bass_utils

F32 = mybir.dt.float32
F32R = mybir.dt.float32r
BF16 = mybir.dt.bfloat16

B_FULL = 8192
N_CORES = 8
B_CORE = B_FULL // N_CORES  # 1024
N = 64
D = 128
OUT_COLS = (N * (N - 1)) // 2  # 2016

BLK = 64                       # samples per input block
BLK_PAIRS = BLK // 2           # 32
GRP = 256                      # samples per group (back-half unit)
GRP_PAIRS = GRP // 2           # 128  (= partition count for j-transpose)
BLKS_PER_GRP = GRP // BLK      # 4


def _tri(i: int) -> int:
    return (i * (i - 1)) // 2


def build_nc(b_core: int = B_CORE, repeats: int = 1, mode: str = "bf16",
             in_eng: str = "gpsimd", balance: bool = True,
             gram_merge: bool = False,
             **_ignored):
    """Build the v2 Bass program for one core processing b_core samples.

    repeats > 1 wraps the whole workload in a hardware loop (timing only).
    in_eng: engine issuing the input DMAs ("gpsimd" = SWDGE, "sync" = HWDGE).
    balance: spread copies across DVE/Act/Pool (False = v2.0 assignment).
    """
    assert b_core % GRP == 0
    n_grp = b_core // GRP
    mm_dt = BF16 if mode == "bf16" else F32R

    nc = bacc.Bacc("TRN2", target_bir_lowering=False, debug=False,
                   num_devices=N_CORES)
    x = nc.dram_tensor("x", [b_core, N, D], F32, kind="ExternalInput").ap()
    ident = nc.dram_tensor("ident", [128, 128], F32, kind="ExternalInput").ap()
    out = nc.dram_tensor("out", [b_core, OUT_COLS], F32,
                         kind="ExternalOutput").ap()

    with tile.TileContext(nc) as tc:
        with (
            tc.tile_pool(name="xin", bufs=3) as xin_pool,
            tc.tile_pool(name="xt", bufs=3) as xt_pool,
            tc.tile_pool(name="zbig", bufs=2) as zbig_pool,
            tc.tile_pool(name="zs", bufs=2) as zs_pool,
            tc.tile_pool(name="zp", bufs=2) as zp_pool,
            tc.tile_pool(name="const", bufs=1) as const_pool,
            tc.tile_pool(name="pst", bufs=3, space=bass.MemorySpace.PSUM) as pst_pool,
            tc.tile_pool(name="psz", bufs=3, space=bass.MemorySpace.PSUM) as psz_pool,
            tc.tile_pool(name="ztp", bufs=2, space=bass.MemorySpace.PSUM) as ztp_pool,
        ):
            ident_sb = const_pool.tile([128, 128], F32)
            nc.sync.dma_start(ident_sb[:], ident[:])
            ident_mm = const_pool.tile([128, 128], mm_dt)
            nc.vector.tensor_copy(ident_mm[:], ident_sb[:])

            def copy_with(eng, dst, src):
                if eng is nc.scalar:
                    eng.copy(dst, src)
                else:
                    eng.tensor_copy(dst, src)

            def front_block(base, blk, zbig):
                """Load + transpose + gram + extract for one 64-sample block."""
                s0 = base + blk * BLK
                src = x[s0:s0 + BLK]
                src = src.rearrange("(c two) n d -> (two n) c d", two=2)
                xin = xin_pool.tile([128, BLK_PAIRS * D], mm_dt)
                dst3 = xin[:].rearrange("p (c d) -> p c d", c=BLK_PAIRS)
                in_engine = nc.gpsimd if in_eng == "gpsimd" else nc.sync
                # two half-block DMAs: halves the DMA->transpose latency
                half = BLK_PAIRS // 2
                in_engine.dma_start(dst3[:, 0:half], src[:, 0:half])
                in_engine.dma_start(dst3[:, half:], src[:, half:])

                xt = xt_pool.tile([128, BLK_PAIRS * D], mm_dt)
                # transposes: 8 chunks per PSUM tile
                for t in range(4):
                    pst = pst_pool.tile([128, 1024], mm_dt)
                    for k in range(8):
                        c = t * 8 + k
                        nc.tensor.transpose(
                            pst[:, k * 128:(k + 1) * 128],
                            xin[:, c * D:(c + 1) * D],
                            ident_mm[:],
                        )
                    nc.vector.tensor_copy(
                        xt[:, t * 1024:(t + 1) * 1024], pst[:])

                # grams (non-transpose matmul output must be fp32; the
                # extract copy casts down to mm_dt)
                if gram_merge:
                    # one [128,128] matmul per pair (half the Ldweights);
                    # off-diagonal 64x64 blocks are cross-sample garbage, so
                    # extraction needs two half-partition copies per tile.
                    for t in range(8):
                        psz = psz_pool.tile([128, 512], F32)
                        for k in range(4):
                            c = t * 4 + k
                            nc.tensor.matmul(
                                psz[:, k * 128:(k + 1) * 128],
                                xt[:, c * 128:(c + 1) * 128],
                                xt[:, c * 128:(c + 1) * 128],
                                start=True, stop=True,
                            )
                        q0 = blk * BLK_PAIRS + t * 4
                        psz4 = psz[:].rearrange("p (k v) -> p k v", k=4)
                        zb = zbig[:, q0 * 64:(q0 + 4) * 64]
                        dstA = zb[0:64].rearrange("p (k v) -> p k v", k=4)
                        dstB = zb[64:128].rearrange("p (k v) -> p k v", k=4)
                        e1 = nc.scalar if t % 2 == 0 else nc.vector
                        e2 = nc.vector if t % 2 == 0 else nc.scalar
                        copy_with(e1, dstA, psz4[0:64, :, 0:64])
                        copy_with(e2, dstB, psz4[64:128, :, 64:128])
                else:
                    # 8 pairs per PSUM tile, 2 matmuls per pair (a halves)
                    for t in range(4):
                        psz = psz_pool.tile([128, 512], F32)
                        for k in range(8):
                            c = t * 8 + k
                            for a in range(2):
                                col = c * 128 + a * 64
                                nc.tensor.matmul(
                                    psz[a * 64:(a + 1) * 64,
                                        k * 64:(k + 1) * 64],
                                    xt[:, col:col + 64],
                                    xt[:, col:col + 64],
                                    start=True, stop=True,
                                )
                        q0 = blk * BLK_PAIRS + t * 8
                        if balance:
                            ext_eng = nc.scalar if t % 2 == 0 else nc.vector
                        else:
                            ext_eng = nc.vector
                        copy_with(ext_eng,
                                  zbig[:, q0 * 64:(q0 + 8) * 64], psz[:])

            def back_quarter(base, qw, quarter, zbig, zs, zp, nq=4):
                """j-transposes + zs copies for one j-chunk, then the pack
                copies that become ready, and (final part) out DMAs.

                qw = pair count of this group (128 full / 64 tail).
                nq = number of parts the group's back half is split into."""
                per = 16 // nq
                zbig_v = zbig[:, 0:qw * 64].rearrange("p (q j) -> p q j", j=64)
                for jt in range(quarter * per, quarter * per + per):
                    ztp = ztp_pool.tile([128, 512], mm_dt)
                    for jj in range(4):
                        j = jt * 4 + jj
                        nc.tensor.transpose(
                            ztp[0:qw, jj * 128:(jj + 1) * 128],
                            zbig_v[:, :, j:j + 1],
                            ident_mm[:],
                        )
                    if balance:
                        zs_eng = nc.vector if jt % 2 == 0 else nc.scalar
                    else:
                        zs_eng = nc.scalar
                    copy_with(zs_eng,
                              zs[0:qw, jt * 512:(jt + 1) * 512], ztp[0:qw, :])
                # pack rows whose j-range is now complete:
                # after quarter q, zs holds j < 16*(q+1), so rows
                # i <= 16*(q+1) are ready.
                zs_v = zs[0:qw].rearrange("p (j a n) -> p j a n", j=64, a=2)
                zp_v = zp[0:qw].rearrange("p (a v) -> p a v", a=2)
                jw = 4 * per
                i_lo = jw * quarter + 1
                i_hi = min(jw * (quarter + 1), N - 1)
                for i in range(i_lo, i_hi + 1):
                    srci = zs_v[:, 0:i, :, i:i + 1]
                    srci = srci.rearrange("p j a n -> p a j n")
                    t0 = _tri(i)
                    dsti = zp_v[:, :, t0:t0 + i].unsqueeze(3)
                    if balance:
                        eng = (nc.vector, nc.gpsimd, nc.scalar)[i % 3]
                    else:
                        eng = nc.vector if i % 2 == 0 else nc.gpsimd
                    copy_with(eng, dsti, srci)
                # column-split output: rows packed by the end of the
                # second-to-last part ship early, overlapping the final
                # pack copies; only the last rows' columns ship at the end.
                split = _tri(jw * (nq - 1) + 1)
                outv = out[base:base + 2 * qw]
                outv = outv.rearrange("(q two) v -> two q v", two=2)
                if quarter == nq - 2:
                    for a in range(2):
                        nc.sync.dma_start(
                            outv[a, :, 0:split], zp_v[:, a, 0:split])
                elif quarter == nq - 1:
                    for a in range(2):
                        nc.sync.dma_start(
                            outv[a, :, split:], zp_v[:, a, split:])

            def body(_iv=None):
                # software-pipelined: front(g) interleaved with back(g-1).
                # The last 256 samples run as two 64-pair groups so the
                # drain (back half with no front to hide behind) is halved.
                group_list = [(b, 128) for b in range(0, b_core, GRP)]
                prev_backs = []
                for base, qw in group_list:
                    nblk = (qw * 2) // BLK
                    zbig = zbig_pool.tile([128, GRP_PAIRS * 64], mm_dt)
                    zs = zs_pool.tile([128, 64 * 128], mm_dt)
                    zp = zp_pool.tile([128, 2 * OUT_COLS], F32)

                    def mk_back(base=base, qw=qw, zbig=zbig, zs=zs, zp=zp):
                        nq = 4 if qw == GRP_PAIRS else 2
                        return [
                            (lambda q=q, base=base, qw=qw, zbig=zbig,
                                    zs=zs, zp=zp, nq=nq:
                             back_quarter(base, qw, q, zbig, zs, zp, nq=nq))
                            for q in range(nq)
                        ]
                    fronts = [
                        (lambda blk=blk, base=base, zbig=zbig:
                         front_block(base, blk, zbig))
                        for blk in range(nblk)
                    ]
                    for idx in range(max(nblk, len(prev_backs))):
                        if idx < len(prev_backs):
                            prev_backs[idx]()
                        if idx < nblk:
                            fronts[idx]()
                    prev_backs = mk_back()
                for th in prev_backs:
                    th()

            if repeats == 1:
                body()
            else:
                with tc.For_i(0, repeats, 1) as _i:
                    body(_i)

    nc.compile()
    return nc


# ---------------------------------------------------------------------------
# v1 (legacy) builder — proven fallback
# ---------------------------------------------------------------------------

def build_nc_legacy(b_core: int = B_CORE, repeats: int = 1, mode: str = "f32r",
                    skip_out: bool = False, ob: int = 256,
                    dma_cast: bool = True, out_only: bool = False,
                    interleave: bool = True):
    OB = ob
    OB_PAIRS = OB // 2
    BLKS_PER_OB = OB // BLK
    assert b_core % OB == 0
    n_ob = b_core // OB
    nc = bacc.Bacc("TRN2", target_bir_lowering=False, debug=False,
                   num_devices=N_CORES)
    x = nc.dram_tensor("x", [b_core, N, D], F32, kind="ExternalInput").ap()
    ident = nc.dram_tensor("ident", [128, 128], F32, kind="ExternalInput").ap()
    out = nc.dram_tensor("out", [b_core, OUT_COLS], F32,
                         kind="ExternalOutput").ap()

    bf16 = mode == "bf16"
    mm_dt = mybir.dt.bfloat16 if bf16 else F32R

    with tile.TileContext(nc) as tc:
        with (
            tc.tile_pool(name="xin", bufs=2) as xin_pool,
            tc.tile_pool(name="xbf", bufs=2) as xbf_pool,
            tc.tile_pool(name="xt", bufs=4) as xt_pool,
            tc.tile_pool(name="zbig", bufs=2 if ob <= 512 else 1) as zbig_pool,
            tc.tile_pool(name="const", bufs=1) as const_pool,
            tc.tile_pool(name="pst", bufs=2, space=bass.MemorySpace.PSUM) as pst_pool,
            tc.tile_pool(name="psz", bufs=2, space=bass.MemorySpace.PSUM) as psz_pool,
        ):
            ident_sb = const_pool.tile([128, 128], F32)
            nc.sync.dma_start(ident_sb[:], ident[:])
            ident_mm = const_pool.tile([128, 128], mm_dt)
            nc.vector.tensor_copy(ident_mm[:], ident_sb[:])

            def emit_out_dmas(zbig, obi):
                outv = out[obi * OB:(obi + 1) * OB]
                outv = outv.rearrange("(q a) v -> a q v", a=2)
                thunks = []
                for i in range(1, N):
                    t0 = _tri(i)
                    for a in range(2):
                        def go(i=i, a=a, t0=t0, zbig=zbig, outv=outv):
                            srcz = zbig[64 * a + i: 64 * a + i + 1]
                            srcz = srcz.rearrange("p (q j) -> p q j", j=N)
                            if i >= 48:
                                eng = nc.gpsimd
                            else:
                                eng = nc.sync if (i + a) % 2 == 0 else nc.scalar
                            eng.dma_start(
                                outv[a, :, t0:t0 + i].unsqueeze(0),
                                srcz[:, :, 0:i],
                            )
                        thunks.append(go)
                return thunks

            def body(_iv=None):
                pending = []
                for obi in range(n_ob):
                    zbig = zbig_pool.tile([128, OB_PAIRS * N], F32)
                    if out_only:
                        nc.gpsimd.memset(zbig[:], 0.0)
                        for th in emit_out_dmas(zbig, obi):
                            th()
                        continue
                    chunk = (len(pending) + BLKS_PER_OB - 1) // BLKS_PER_OB \
                        if pending else 0
                    for blk in range(BLKS_PER_OB):
                        s0 = obi * OB + blk * BLK
                        src = x[s0:s0 + BLK]
                        src = src.rearrange("(c two) n d -> (two n) c d", two=2)
                        if dma_cast:
                            xsrc = xbf_pool.tile([128, BLK_PAIRS * D], mm_dt)
                            dst3 = xsrc[:].rearrange("p (c d) -> p c d",
                                                     c=BLK_PAIRS)
                            nc.gpsimd.dma_start(dst3, src)
                        else:
                            xin = xin_pool.tile([128, BLK_PAIRS * D], F32)
                            dst3 = xin[:].rearrange("p (c d) -> p c d",
                                                    c=BLK_PAIRS)
                            nc.gpsimd.dma_start(dst3, src)
                            xsrc = xbf_pool.tile([128, BLK_PAIRS * D], mm_dt)
                            nc.vector.tensor_copy(xsrc[:], xin[:])

                        for grp in range(BLK_PAIRS // 4):
                            pst = pst_pool.tile([128, 512], mm_dt)
                            xt = xt_pool.tile([128, 512], mm_dt)
                            for k in range(4):
                                c = grp * 4 + k
                                nc.tensor.transpose(
                                    pst[:, k * 128:(k + 1) * 128],
                                    xsrc[:, c * D:(c + 1) * D].bitcast(mm_dt),
                                    ident_mm[:].bitcast(mm_dt),
                                )
                            nc.vector.tensor_copy(xt[:], pst[:])

                            psz = psz_pool.tile([128, 1024], F32)
                            for k in range(4):
                                lhsT = xt[:, k * 128:(k + 1) * 128]
                                g2 = (k // 2) * 256
                                rhs = xt[:, g2:g2 + 256]
                                off = k * 256 - (k % 2) * 128
                                nc.tensor.matmul(
                                    psz[:, off:off + 256], lhsT, rhs,
                                    start=True, stop=True,
                                )
                            psz4 = psz[:].rearrange("p (k v) -> p k v", k=4)
                            qq0 = (blk * BLK_PAIRS + grp * 4) * N
                            dst = zbig[:, qq0:qq0 + 256]
                            dstA = dst[0:64].rearrange("p (k v) -> p k v", k=4)
                            dstB = dst[64:128].rearrange("p (k v) -> p k v", k=4)
                            nc.scalar.copy(dstA, psz4[0:64, :, 0:64])
                            nc.scalar.copy(dstB, psz4[64:128, :, 64:128])
                        if pending:
                            for th in pending[:chunk]:
                                th()
                            pending = pending[chunk:]
                    if pending:
                        for th in pending:
                            th()
                        pending = []
                    if skip_out:
                        flat = zbig[:, 0:OUT_COLS * 2]
                        dstf = out[obi * OB:obi * OB + 256]
                        dstv = dstf.rearrange("(p r) v -> p (r v)", p=128)
                        nc.sync.dma_start(dstv, flat)
                        continue
                    if interleave and obi < n_ob - 1:
                        pending = emit_out_dmas(zbig, obi)
                    else:
                        for th in emit_out_dmas(zbig, obi):
                            th()

            if repeats == 1:
                body()
            else:
                with tc.For_i(0, repeats, 1) as _i:
                    body(_i)

    nc.compile()
    return nc


_CACHED = {"nc": None, "cfg": None}

# (builder, mode) in preference order; later entries are fallbacks in case a
# config fails compile/verification in the target environment.
_CONFIGS = [
    ("v2", "bf16"),
    ("legacy", "f32r"),
    ("legacy", "bf16"),
]


def kernel(inputs: np.ndarray) -> np.ndarray:
    """Full-input entry point: inputs [8192, 64, 128] fp32 -> [8192, 2016]."""
    inputs = np.ascontiguousarray(np.asarray(inputs, dtype=np.float32))
    assert inputs.shape == (B_FULL, N, D), inputs.shape
    ident = np.eye(128, dtype=np.float32)
    in_maps = [
        {"x": inputs[c * B_CORE:(c + 1) * B_CORE], "ident": ident}
        for c in range(N_CORES)
    ]
    if _CACHED["nc"] is not None:
        res = bass_utils.run_bass_kernel_spmd(
            _CACHED["nc"], in_maps, core_ids=list(range(N_CORES)))
        return np.concatenate([r["out"] for r in res.results], axis=0)
    last_err = None
    for builder, mode in _CONFIGS:
        try:
            if builder == "v2":
                nc = build_nc(mode=mode)
            else:
                nc = build_nc_legacy(mode=mode)
            res = bass_utils.run_bass_kernel_spmd(
                nc, in_maps, core_ids=list(range(N_CORES)))
            _CACHED["nc"] = nc
            _CACHED["cfg"] = (builder, mode)
            return np.concatenate([r["out"] for r in res.results], axis=0)
        except Exception as e:  # compile/verifier failure -> next config
            last_err = e
    raise last_err
